# revision 55
# baseline (speedup 1.0000x reference)
"""Segment-mean + projection kernel for Trainium2 (8 NeuronCores, SPMD).

logits[b] = (mean of x rows in bag b) @ rel_weight.T + bias

Strategy: data-parallel over bags.
- Host: x rows are pre-scaled by 1/count and quantized to fp8 e4m3 (so the
  device segment-SUM directly yields means); bags with count <= RESID_T get
  fp8 residual rows appended (same bag id), recovering near-fp16 accuracy
  for the small bags that dominate the max-error while keeping one uniform
  fp8 pipeline. Bags are packed whole into 768-row groups (<= NB bags each)
  so no cross-group fixup is needed; row pairs are elementwise-interleaved
  so DoubleRow fp8 matmuls stream 2 contraction rows per cycle.
- Device, per group: the DVE builds all six 128-row one-hot tiles in one
  tensor_tensor (broadcast APs); the PE accumulates means [bag, D] with 6
  DoubleRow fp8 matmuls (3 pairs x 2 psum halves); ACT+DVE copy PSUM->SBUF
  fp16 (split across engines so their psum-stop waits overlap); the PE
  transposes the means via plain matmuls against identity (N=NB) and
  projects against W.T chunks with fp32 psum accumulation; ACT adds bias
  and results are batched out OUT_BATCH groups per DMA.
- DMA: x flows in 4-group quads rotated over the three DMA paths
  (sync-HWDGE / scalar-HWDGE / gpsimd-SWDGE), with a fine-grained ramp for
  the first 12 groups; all f16 constants ship in a single packed DMA; the
  output is 128-partition-shaped so descriptors spread over all SDMA
  engines.
"""
import sys
import re

sys.path.insert(0, "/opt/trn_rl_repo")

import numpy as np
import ml_dtypes

F8 = ml_dtypes.float8_e4m3  # matches mybir.dt.float8e4

N_CORES = 8
TILES = 6
RPG = 768  # rows per group
D = 690
DP = 704  # padded D = 5*128 + 64
CHUNKW = [128, 128, 128, 128, 128, 64]
C = 53
NB = 96  # max bags per group (the one-hot width)
RESID_T = 2  # bags with count <= T get fp8 residual rows
USE_DR = True  # DoubleRow fp8 matmuls (2 contraction rows / cycle)
OUT_BATCH = 4  # groups per output DMA


def _apply_walrus_workarounds():
    """This walrus build allows at most one semaphore wait per instruction
    on several opcodes (Drain, Matmult/LDW). Patch Tile's tail drain to use
    standalone wait_ge instructions, and provide a post-pass that hoists
    excess waits onto InstNoOp instructions."""
    from concourse import tile, mybir

    def _patched_drain_and_barrier(self, tick_clock, wait_clock):
        gc = tick_clock.global_clock
        ticks = [int(s) for s in re.findall(r"\d+", repr(gc))]
        allocated = self.sems.allocated()
        for proc, sem in sorted(allocated.items()):
            t = ticks[proc] if proc < len(ticks) else 0
            if t > 0:
                mult = 16 if "DMA" in sem.name else 1
                self.nc.sync.wait_ge(sem, t * mult)
        self.nc.sync.drain()
        self.nc.all_engine_barrier()
        popped = self.nc._tile_sem_poison_stack.pop()
        assert popped is self._sem_poison
        self.nc.clear_and_free_semaphores(list(allocated.values()))
        self.nc.all_engine_barrier()

    tile.TileContext._drain_and_barrier = _patched_drain_and_barrier

    def split_multi_waits(nc, max_waits=1):
        for f in nc.m.functions:
            for b in f.blocks:
                insts = list(b.instructions)
                new = []
                dirty = False
                for inst in insts:
                    si = inst.sync_info
                    if si is not None and len(si.on_wait) > max_waits:
                        waits = list(si.on_wait)
                        extra, keep = waits[:-max_waits], waits[-max_waits:]
                        for k, w in enumerate(extra):
                            nop = mybir.InstNoOp(
                                name=f"{inst.name}-hw{k}", ins=[], outs=[]
                            )
                            nop.engine = inst.engine
                            nop.sync_info = mybir.SyncInfo(
                                on_wait=[w], on_update=[]
                            )
                            new.append(nop)
                        inst.sync_info = mybir.SyncInfo(
                            on_wait=keep, on_update=list(si.on_update)
                        )
                        dirty = True
                    new.append(inst)
                if dirty:
                    b.instructions = new

    return split_multi_waits


def _preprocess(x, scope, n_cores=N_CORES):
    """Quantize + pack. Returns per-core input tensors and assembly maps."""
    n_sent = x.shape[0]
    n_bags = scope.shape[0] - 1
    scope = np.asarray(scope, dtype=np.int64)
    counts = np.diff(scope)
    assert counts.min() >= 1
    seg_full = np.repeat(np.arange(n_bags, dtype=np.int64), counts)

    # pre-scale rows by 1/count, quantize to fp8; residuals for small bags
    xs = x / counts[seg_full][:, None].astype(np.float32)
    q1 = xs.astype(F8)
    small = counts <= RESID_T
    small_rows = small[seg_full]
    q2 = (xs - q1.astype(np.float32)).astype(F8)

    r_eff = counts * (1 + small.astype(np.int64))
    assert r_eff.max() <= RPG

    # contiguous bag spans per core, balanced by effective rows
    cum = np.cumsum(r_eff)
    total = int(cum[-1])
    bag_cuts = [0]
    for k in range(1, n_cores):
        bag_cuts.append(int(np.searchsorted(cum, total * k / n_cores)))
    bag_cuts.append(n_bags)

    # greedy-pack whole bags into groups per core
    core_groups = []  # per core: list of (first_bag, n_bags_in_group)
    for c in range(n_cores):
        b0, b1 = bag_cuts[c], bag_cuts[c + 1]
        groups = []
        gb0, rows, nb = b0, 0, 0
        for b in range(b0, b1):
            rb = int(r_eff[b])
            if rows + rb > RPG or nb >= NB:
                groups.append((gb0, nb))
                gb0, rows, nb = b, 0, 0
            rows += rb
            nb += 1
        if nb:
            groups.append((gb0, nb))
        core_groups.append(groups)

    G = max(len(g) for g in core_groups)
    G += G % 2  # even, for paired-group DMAs

    cores = []
    for c in range(n_cores):
        groups = core_groups[c]
        nb_g = np.zeros(G, dtype=np.int64)
        base_g = np.zeros(G, dtype=np.int64)
        # destination row of each bag's first row
        bag_dest = np.zeros(n_bags + 1, dtype=np.int64)
        bag_local = np.zeros(n_bags, dtype=np.int64)
        for g, (gb0, nb) in enumerate(groups):
            nb_g[g] = nb
            base_g[g] = gb0
            ptr = g * RPG
            for i in range(nb):
                b = gb0 + i
                bag_dest[b] = ptr
                bag_local[b] = i
                ptr += int(r_eff[b])

        b0, b1 = bag_cuts[c], bag_cuts[c + 1]
        r0, r1 = int(scope[b0]), int(scope[b1])
        seg_c = seg_full[r0:r1]
        within = np.arange(r0, r1) - scope[seg_c]
        dest1 = bag_dest[seg_c] + within
        rows_small = small_rows[r0:r1]
        dest2 = (bag_dest[seg_c] + counts[seg_c] + within)[rows_small]

        x_rows = np.zeros((G * RPG, DP), dtype=F8)
        x_rows[dest1, :D] = q1[r0:r1]
        x_rows[dest2, :D] = q2[r0:r1][rows_small]
        seg_local = np.full(G * RPG, 128.0, dtype=np.float16)
        seg_local[dest1] = bag_local[seg_c]
        seg_local[dest2] = bag_local[seg_c][rows_small]

        # x layout: [G, pair(3), i(2), p(128), DP] -> [Gp/2*128, 2*3*DP*2]
        # pair rows elementwise-interleaved (i innermost) so DoubleRow can
        # stream 2 contraction rows per cycle; two groups share one DMA row.
        x_dram = np.ascontiguousarray(
            x_rows.reshape(G, 3, 2, 128, DP).transpose(0, 3, 1, 4, 2)
        ).reshape(G // 2, 2, 128, 3 * DP * 2)
        x_dram = np.ascontiguousarray(
            x_dram.transpose(0, 2, 1, 3)
        ).reshape(G // 2 * 128, 2 * 3 * DP * 2)
        # seg: [G, tile(6), p(128)] -> [128, G*6]
        seg_sb = np.ascontiguousarray(
            seg_local.reshape(G, TILES, 128).transpose(2, 0, 1)
        ).reshape(128, G * TILES)

        cores.append(
            dict(x=x_dram, seg=seg_sb, nb_g=nb_g, base_g=base_g)
        )
    return cores, G


def _build_program(G):
    import concourse.bass as bass
    import concourse.mybir as mybir
    from concourse import tile

    dt = mybir.dt
    nc = bass.Bass()
    DR = mybir.MatmulPerfMode.DoubleRow if USE_DR else None

    x_d = nc.declare_dram_parameter(
        "x", [G // 2 * 128, 2 * TILES * DP], dt.float8e4, isOutput=False
    )
    # all f16 constants in one tensor: [iota | ident | wt | seg]
    NCONST = 128 + 128 + TILES * 128 + G * TILES
    const_d = nc.declare_dram_parameter(
        "consts", [128, NCONST], dt.float16, isOutput=False
    )
    bias_d = nc.declare_dram_parameter("bias", [128, 1], dt.float32, isOutput=False)
    n_obat = (G + OUT_BATCH - 1) // OUT_BATCH
    # each batch of OUT_BATCH groups is a contiguous [C, OUT_BATCH*128] block
    # so out-DMA descriptors spread across SDMA engines
    out_d = nc.declare_dram_parameter(
        "out", [n_obat * 128, OUT_BATCH * NB], dt.float32, isOutput=True
    )

    with tile.TileContext(nc) as tc:
        with (
            tc.tile_pool(name="const", bufs=1) as cpool,
            tc.tile_pool(name="xin", bufs=5) as xpool,
            tc.tile_pool(name="onehot", bufs=4) as apool,
            tc.tile_pool(name="means", bufs=5) as mpool,
            tc.tile_pool(name="mgt", bufs=4) as tpool,
            tc.tile_pool(name="outs", bufs=3) as opool,
            tc.tile_pool(name="ps_a", bufs=3, space="PSUM") as psapool,
            tc.tile_pool(name="ps_b", bufs=2, space="PSUM") as psbpool,
            tc.tile_pool(name="ps_tr", bufs=1, space="PSUM") as ptpool,
            tc.tile_pool(name="ps_proj", bufs=1, space="PSUM") as pppool,
        ):
            const_t = cpool.tile([128, NCONST], dt.float16)
            bias_t = cpool.tile([128, 1], dt.float32)

            # one packed const DMA on the sync HWDGE ring, ahead of
            # everything else there (it gates the first one-hot build)
            nc.sync.dma_start(out=const_t[:], in_=const_d[:])
            nc.gpsimd.dma_start(out=bias_t[:], in_=bias_d[:])
            iota_t = const_t[:, 0:128]
            ident_t = const_t[:, 128:256]
            wt_t = const_t[:, 256 : 256 + TILES * 128]
            seg_t = const_t[:, 256 + TILES * 128 : NCONST]

            iota_bc = iota_t[:, 0:NB].unsqueeze(1).broadcast_to([128, TILES, NB])

            # software-pipelined: stage k of group g happens at iter g+k
            st = [None, None, None]  # (ps_a, ps_b), means, mgt rolling state
            out_acc = None
            x_half = None
            x_base = 0
            a_tiles = {}
            # startup: fine-grained transfers round-robined over the three
            # DMA paths so compute starts early; then 4-group quads
            dma_plan = {}
            if G >= 12:
                dma_plan[0] = (2, [nc.sync, nc.scalar])
                dma_plan[2] = (2, [nc.gpsimd, nc.sync])
                dma_plan[4] = (2, [nc.scalar, nc.gpsimd])
                dma_plan[6] = (2, [nc.sync])
                dma_plan[8] = (2, [nc.scalar])
                dma_plan[10] = (2, [nc.gpsimd])
                rings = [nc.sync, nc.scalar, nc.gpsimd]
                g = 12
                ridx = 0
                while g < G:
                    # 4-group quads mid-run; the last 8 groups go as 2-group
                    # pairs spread across rings so the tail arrives in
                    # parallel instead of serialized per-ring
                    ng = min(2, G - g)
                    dma_plan[g] = (ng, [rings[ridx % 3]])
                    ridx += 1
                    g += ng
            else:
                for g in range(0, G, 2):
                    dma_plan[g] = (
                        2,
                        [[nc.sync, nc.scalar, nc.gpsimd][(g // 2) % 3]],
                    )

            def build_onehot(g):
                a_t = apool.tile([128, TILES * 128], dt.float8e4, tag="a")
                a_v = a_t[:].rearrange("p (t b) -> p t b", t=TILES)
                if g < 4:
                    # rotating buffers: zero the unused col stripes once;
                    # later builds only write cols 0:NB of each tile
                    nc.vector.memset(a_v[:, :, NB:128], 0)
                seg_bc = (
                    seg_t[:, g * TILES : (g + 1) * TILES]
                    .unsqueeze(2)
                    .broadcast_to([128, TILES, NB])
                )
                nc.vector.tensor_tensor(
                    out=a_v[:, :, 0:NB],
                    in0=iota_bc,
                    in1=seg_bc,
                    op=mybir.AluOpType.is_equal,
                )
                a_tiles[g] = a_t

            for it in range(G + 2):
                st = [None] + st[:2]
                # ---- stage 0: DMA + one-hot + sum matmuls for group g0
                g0 = it
                if g0 < G:
                    GW = TILES * DP  # columns per group in a DMA row
                    if g0 in dma_plan:
                        ng, engs = dma_plan[g0]
                        x2_t = xpool.tile([128, ng * GW], dt.float8e4, tag="x")
                        q = g0 // 2
                        if g0 == 0:
                            # first pair: quarter-group sub-DMAs on both
                            # rings so compute starts ASAP (scalar ring is
                            # const-free and serves the first quarter)
                            hw = GW // 2
                            order = [nc.scalar, nc.sync, nc.scalar, nc.sync]
                            for e_i in range(4):
                                order[e_i].dma_start(
                                    out=x2_t[:, e_i * hw : (e_i + 1) * hw],
                                    in_=x_d[
                                        q * 128 : (q + 1) * 128,
                                        e_i * hw : (e_i + 1) * hw,
                                    ],
                                )
                        elif len(engs) == 2:
                            # two single-group sub-DMAs (fast startup)
                            for e_i, eng in enumerate(engs):
                                eng.dma_start(
                                    out=x2_t[:, e_i * GW : (e_i + 1) * GW],
                                    in_=x_d[
                                        q * 128 : (q + 1) * 128,
                                        e_i * GW : (e_i + 1) * GW,
                                    ],
                                )
                        elif ng == 2:
                            engs[0].dma_start(
                                out=x2_t[:], in_=x_d[q * 128 : (q + 1) * 128, :]
                            )
                        else:
                            # split the quad into two 2-group transfers so
                            # the first half's groups unblock compute sooner
                            engs[0].dma_start(
                                out=x2_t[:, 0 : 2 * GW],
                                in_=x_d[q * 128 : (q + 1) * 128, :],
                            )
                            engs[0].dma_start(
                                out=x2_t[:, 2 * GW : 4 * GW],
                                in_=x_d[(q + 1) * 128 : (q + 2) * 128, :],
                            )
                        x_half = x2_t
                        x_base = g0
                    x_t = x_half[:, (g0 - x_base) * GW : (g0 - x_base + 1) * GW]
                    # one-hot built one iteration ahead so the DVE isn't on
                    # the critical path into the sum matmuls
                    if g0 == 0:
                        build_onehot(0)
                    if g0 + 1 < G:
                        build_onehot(g0 + 1)
                    a_t = a_tiles.pop(g0)
                    ps_a = psapool.tile([128, DP // 2], dt.float32, tag="psa")
                    ps_b = psbpool.tile([128, DP // 2], dt.float32, tag="psb")
                    means = mpool.tile([128, DP], dt.float16, tag="m")
                    # x cols per pair j: d-major, i (k-tile of pair) innermost
                    x4 = x_t.rearrange("p (j d i) -> p j d i", j=3, i=2)
                    a4 = a_t[:].rearrange("p (j i b) -> p j i b", j=3, i=2)  # b=NB
                    H = DP // 2
                    if USE_DR:
                        for j in range(3):
                            nc.tensor.matmul(
                                ps_a[:],
                                a4[:, j],
                                x4[:, j, 0:H, :].transpose([0, 2, 1]),
                                start=(j == 0),
                                stop=(j == 2),
                                perf_mode=DR,
                            )
                            nc.tensor.matmul(
                                ps_b[:],
                                a4[:, j],
                                x4[:, j, H:DP, :].transpose([0, 2, 1]),
                                start=(j == 0),
                                stop=(j == 2),
                                perf_mode=DR,
                            )
                        nc.scalar.activation(
                            means[:, 0:H],
                            ps_a[:],
                            mybir.ActivationFunctionType.Copy,
                        )
                    else:
                        for t in range(TILES):
                            j, i = t // 2, t % 2
                            nc.tensor.matmul(
                                ps_a[:],
                                a4[:, j, i],
                                x4[:, j, 0:H, i],
                                start=(t == 0),
                                stop=(t == TILES - 1),
                            )
                            nc.tensor.matmul(
                                ps_b[:],
                                a4[:, j, i],
                                x4[:, j, H:DP, i],
                                start=(t == 0),
                                stop=(t == TILES - 1),
                            )
                    # b-half means copy on DVE so its psum-stop wait
                    # overlaps the ACT's a-half wait
                    nc.vector.tensor_copy(means[:, H:DP], ps_b[:])
                    st[0] = (means,)

                # ---- stage 1: transpose group g1's means via matmul vs identity
                g1 = it - 1
                if 0 <= g1 < G:
                    (means,) = st[1]
                    # chunk 5 placed so no matmul output crosses the 2KB
                    # PSUM bank boundary (512 f32 cols)
                    P5 = (
                        5 * NB
                        if (5 * NB) // 512 == (6 * NB - 1) // 512
                        else ((5 * NB) // 512 + 1) * 512
                    )
                    ps_t = ptpool.tile([128, 1024], dt.float32, tag="pt")
                    for cch in range(TILES):
                        w = CHUNKW[cch]
                        pcol = cch * NB if cch < 5 else P5
                        nc.tensor.matmul(
                            ps_t[0:w, pcol : pcol + NB],
                            means[:, cch * 128 : cch * 128 + w],
                            ident_t[:, 0:NB],
                            start=True,
                            stop=True,
                        )
                    mgt = tpool.tile([128, TILES * NB], dt.float16, tag="mgt")
                    nc.scalar.activation(
                        mgt[:, 0 : 5 * NB],
                        ps_t[:, 0 : 5 * NB],
                        mybir.ActivationFunctionType.Copy,
                    )
                    nc.vector.tensor_copy(
                        mgt[0:64, 5 * NB : 6 * NB], ps_t[0:64, P5 : P5 + NB]
                    )
                    st[1] = (means, mgt)

                # ---- stage 2: project group g2, add bias, batch out
                g2 = it - 2
                if 0 <= g2 < G:
                    mgt = st[2][1]
                    pp = pppool.tile([128, NB], dt.float32, tag="pp")
                    for cch in range(TILES):
                        w = CHUNKW[cch]
                        nc.tensor.matmul(
                            pp[:],
                            wt_t[0:w, cch * 128 : (cch + 1) * 128],
                            mgt[0:w, cch * NB : (cch + 1) * NB],
                            start=(cch == 0),
                            stop=(cch == TILES - 1),
                        )
                    if g2 % OUT_BATCH == 0:
                        out_acc = opool.tile(
                            [128, OUT_BATCH * NB], dt.float32, tag="o"
                        )
                    col = (g2 % OUT_BATCH) * NB
                    nc.scalar.activation(
                        out_acc[:, col : col + NB],
                        pp[:],
                        mybir.ActivationFunctionType.Identity,
                        bias=bias_t[:],
                    )
                    if g2 % OUT_BATCH == OUT_BATCH - 1 or g2 == G - 1:
                        q = g2 // OUT_BATCH
                        wdt = (g2 % OUT_BATCH) * NB + NB
                        eng = nc.sync if q % 2 == 0 else nc.scalar
                        eng.dma_start(
                            out=out_d[q * 128 : (q + 1) * 128, 0:wdt],
                            in_=out_acc[:, 0:wdt],
                        )
    return nc


def prepare(x, scope, rel_weight, bias):
    """Build the SPMD program + per-core input maps."""
    split_multi_waits = _apply_walrus_workarounds()

    x = np.asarray(x, dtype=np.float32)
    scope_np = np.asarray(scope)
    rel_weight = np.asarray(rel_weight, dtype=np.float32)
    bias = np.asarray(bias, dtype=np.float32)
    n_bags = scope_np.shape[0] - 1

    cores, G = _preprocess(x, scope_np)
    nc = _build_program(G)
    split_multi_waits(nc)

    iota = np.tile(np.arange(128, dtype=np.float16), (128, 1))
    ident = np.eye(128, dtype=np.float16)
    wpad = np.zeros((C, TILES * 128), dtype=np.float32)
    wpad[:, :D] = rel_weight
    wt = np.zeros((128, TILES * 128), dtype=np.float16)
    for cch in range(TILES):
        wt[:, cch * 128 : cch * 128 + C] = wpad[:, cch * 128 : (cch + 1) * 128].T
    const_common = np.concatenate([iota, ident, wt], axis=1)
    bias_in = np.zeros((128, 1), dtype=np.float32)
    bias_in[:C, 0] = bias

    in_maps = []
    for c in range(N_CORES):
        cd = cores[c]
        in_maps.append(
            {
                "x": cd["x"],
                "consts": np.ascontiguousarray(
                    np.concatenate([const_common, cd["seg"]], axis=1)
                ),
                "bias": bias_in,
            }
        )

    n_obat = (G + OUT_BATCH - 1) // OUT_BATCH

    def assemble(results):
        logits_t = np.empty((C, n_bags), dtype=np.float32)
        for c in range(N_CORES):
            ob = results[c]["out"]  # [n_obat*128, OUT_BATCH*128]
            out = np.concatenate(
                [ob[q * 128 : q * 128 + C] for q in range(n_obat)], axis=1
            )  # [C, G*128] (possibly padded past G*128)
            cd = cores[c]
            for g in range(G):
                nb = int(cd["nb_g"][g])
                if nb == 0:
                    continue
                b0 = int(cd["base_g"][g])
                logits_t[:, b0 : b0 + nb] = out[:, g * NB : g * NB + nb]
        return np.ascontiguousarray(logits_t.T)

    return dict(nc=nc, in_maps=in_maps, assemble=assemble, G=G)


def kernel(x, scope, rel_weight, bias):
    from concourse.bass_utils import run_bass_kernel_spmd

    p = prepare(x, scope, rel_weight, bias)
    res = run_bass_kernel_spmd(p["nc"], p["in_maps"], list(range(N_CORES)))
    return p["assemble"](res.results)
